# revision 1
# baseline (speedup 1.0000x reference)
"""MemoryBank scatter-gather kernel for 8 Trainium2 NeuronCores.

Reference (per token n of 2048, K=500 neighbor slots padded with index 0):
    neigh = l2norm(wordmem[idx[n]]); q = l2norm(word_embs[n])
    score = q @ neigh.T
    attn  = exp(score) * (k < len) / sum(...)
    out   = attn @ bankmem[idx[n]]

Strategy (tokens data-parallel, 256/core; tables replicated in HBM):
  * All row fetches use the calibrated InstDMAGatherAnt (nc.gpsimd.dma_gather):
    int16 indices force splitting the 100000-row tables into 4 range banks of
    25000 rows; each token's valid neighbors (k < len) are bank-sorted on the
    host.  Tables are host-padded to 256B-multiple rows (gather constraint):
    wordmem -> l2-normalized [100000, 128] f32, bankmem -> [100000, 448] f32.
    l2norm(gather(w)) == gather(l2norm(w)), so normalizing the table on the
    host once replaces per-gathered-row norms on device.
  * Tokens are length-sorted into 16 blocks of 128; core c runs blocks
    (c, 15-c) so per-core work balances.  Per block-slot s and bank b the
    score-path budget BW[s][b] = max neighbor count (tight, data-dependent;
    the Bass program is rebuilt & cached per budget signature).
  * Score path: tokens on partitions.  dma_gather writes [p, j] = w-hat row of
    token p's j-th sorted neighbor; DVE mul+reduce against q-hat gives scores;
    shipped maskneg (-1e9 at pads) + ACT Exp with accum_out -> softmax.
  * Output path: k on partitions.  attn (normalized in token layout) is
    re-laid into the 128-aligned bank-chunk axis, PE-transposed, and written
    onto a persistent zeroed [128,64,64] block-diagonal tile; each real
    output column (token, bank, k-chunk) is one [128,1-hot]x[128,448] f32r
    matmul PSUM-accumulated per 64-token group.  Columns that are entirely
    padding are skipped at build time (no gather, no matmul).
"""

import os

import numpy as np

import concourse.bacc as bacc
import concourse.bass as bass
import concourse.mybir as mybir
import concourse.tile as tile
from concourse.bass_utils import run_bass_kernel_spmd
from concourse.masks import make_identity

NUM = 2048
K = 500
WD = 100
HD = 400
V = 100000
N_CORES = 8
TPC = NUM // N_CORES           # 256 tokens per core
NBLK = TPC // 128              # 2 block slots per core
NBANK = 4
BROWS = V // NBANK             # 25000 rows per bank (< int16 max)
WDP = 128                      # padded w-hat row elems (512B)
HDP = 448                      # padded bank row elems (1792B)
NEG = -1.0e9

MW = int(os.environ.get("MB_MW", "32"))    # score-gather cols per instruction
MBC = int(os.environ.get("MB_MBC", "16"))  # bank-gather cols per instruction
MM_DT = os.environ.get("MB_MM_DT", "f32r")

_CACHE: dict = {}


# --------------------------------------------------------------------------
# host planning
# --------------------------------------------------------------------------

def _plan(idx, lengths):
    """Sort tokens by length into blocks, bank-sort each token's neighbors,
    and compute per-slot budgets + the shared output column stream."""
    idx = np.asarray(idx, dtype=np.int32)
    lengths = np.asarray(lengths, dtype=np.int64)

    order = np.argsort(lengths, kind="stable")
    blocks = order.reshape(16, 128)                    # 16 length-sorted blocks
    # core c gets blocks (c, 15-c): slot 0 = blocks 0..7, slot 1 = 15..8
    core_blocks = [(c, 15 - c) for c in range(N_CORES)]

    # per token: bank-sorted local indices + per-bank counts
    loc = [None] * NUM
    counts = np.zeros((NUM, NBANK), dtype=np.int64)
    for t in range(NUM):
        v = idx[t, : lengths[t]]
        b = v // BROWS
        o = np.argsort(b, kind="stable")
        sv = v[o]
        sb = b[o]
        loc[t] = (sv - sb * BROWS).astype(np.int16)
        counts[t] = np.bincount(b, minlength=NBANK)

    # slot budgets: BW tight ints; per-(slot, bank, token-pos) chunk counts
    plan = {"core_blocks": core_blocks, "blocks": blocks, "loc": loc,
            "counts": counts, "slots": []}
    for s in range(NBLK):
        blks = [core_blocks[c][s] for c in range(N_CORES)]
        toks = blocks[blks]                            # [8, 128] token ids
        cnt = counts[toks]                             # [8, 128, 4]
        BW = cnt.max(axis=(0, 1))                      # [4]
        SW = np.concatenate([[0], np.cumsum(BW)])
        KPW = int(SW[-1])
        # per token-position max chunk count across cores, per bank
        posmax = cnt.max(axis=0)                       # [128, 4]
        nchunk = -(-posmax // 128)                     # ceil
        BO = -(-BW // 128) * 128
        SO = np.concatenate([[0], np.cumsum(BO)])
        KPO = int(SO[-1])
        # shared output column stream: (bank, G, c, tt) for real columns
        stream = []
        for b in range(NBANK):
            for g in range(2):
                for cc in range(int(BO[b]) // 128):
                    for tt in range(64):
                        if nchunk[g * 64 + tt, b] > cc:
                            stream.append((b, g, cc, tt))
        plan["slots"].append({
            "BW": BW, "SW": SW, "KPW": KPW,
            "BO": BO, "SO": SO, "KPO": KPO,
            "nchunk": nchunk, "stream": stream,
        })
    return plan


def _wrap16(flat_i16):
    """i-th index -> (i%16, i//16), replicated to 128 partitions."""
    n = flat_i16.shape[0]
    assert n % 16 == 0
    blk = flat_i16.reshape(-1, 16).T.copy()            # [16, n/16]
    return np.tile(blk, (8, 1))                        # [128, n/16]


def _per_core_arrays(plan, we_hat, core):
    """Build one core's input arrays for both slots."""
    out = {}
    for s in range(NBLK):
        sl = plan["slots"][s]
        blk = plan["blocks"][plan["core_blocks"][core][s]]   # [128] token ids
        KPW, SW, BW = sl["KPW"], sl["SW"], sl["BW"]

        q2 = np.zeros((128, WDP), dtype=np.float32)
        q2[:, :WD] = we_hat[blk]

        # score-path indices [128 tok, KPW] bank-segmented, 0-padded
        iw = np.zeros((128, KPW), dtype=np.int16)
        mneg = np.full((128, max(KPW, 1)), NEG, dtype=np.float32)
        for p, t in enumerate(blk):
            lt = plan["loc"][t]
            ct = plan["counts"][t]
            off = 0
            for b in range(NBANK):
                c = int(ct[b])
                iw[p, SW[b] : SW[b] + c] = lt[off : off + c]
                mneg[p, SW[b] : SW[b] + c] = 0.0
                off += c
        # wrap each bank segment independently (per-bank gather instructions)
        iw16 = np.concatenate(
            [_wrap16(iw[:, SW[b] : SW[b + 1]].T.ravel()) for b in range(NBANK)
             if BW[b] > 0],
            axis=1,
        ) if KPW else np.zeros((128, 0), np.int16)

        # output-path indices: shared column stream
        stream = sl["stream"]
        ib = np.zeros((128, len(stream)), dtype=np.int16)
        for j, (b, g, cc, tt) in enumerate(stream):
            t = blk[g * 64 + tt]
            lt = plan["loc"][t]
            ct = plan["counts"][t]
            off = int(ct[:b].sum())
            lo = cc * 128
            hi = min(lo + 128, int(ct[b]))
            if hi > lo:
                ib[lo - lo : hi - lo, j] = lt[off + lo : off + hi]
        # wrap per bank segment of the stream
        segs = []
        j0 = 0
        for b in range(NBANK):
            nb = sum(1 for (bb, *_r) in stream if bb == b)
            if nb:
                segs.append(_wrap16(ib[:, j0 : j0 + nb].T.ravel()))
            j0 += nb
        ib16 = (np.concatenate(segs, axis=1)
                if segs else np.zeros((128, 0), np.int16))

        out[f"q{s}"] = q2
        out[f"mneg{s}"] = mneg
        out[f"iw{s}"] = iw16
        out[f"ib{s}"] = ib16
    return out


# --------------------------------------------------------------------------
# bass program (built per budget signature)
# --------------------------------------------------------------------------

def _build_nc(plan):
    nc = bacc.Bacc(None, target_bir_lowering=False)
    mm_dt = mybir.dt.float32r if MM_DT == "f32r" else mybir.dt.float32
    bm_dt = mm_dt

    wm_d = nc.dram_tensor("wm", [V, WDP], mybir.dt.float32, kind="ExternalInput")
    bm_d = nc.dram_tensor("bm", [V, HDP], bm_dt, kind="ExternalInput")
    doc_d = nc.dram_tensor("doc", [TPC, HDP], mybir.dt.float32,
                           kind="ExternalOutput")
    slot_in = []
    for s in range(NBLK):
        sl = plan["slots"][s]
        nstream = len(sl["stream"])
        slot_in.append({
            "q": nc.dram_tensor(f"q{s}", [128, WDP], mybir.dt.float32,
                                kind="ExternalInput"),
            "mneg": nc.dram_tensor(f"mneg{s}", [128, max(sl["KPW"], 1)],
                                   mybir.dt.float32, kind="ExternalInput"),
            "iw": nc.dram_tensor(f"iw{s}", [128, 8 * sl["KPW"]],
                                 mybir.dt.int16, kind="ExternalInput"),
            "ib": nc.dram_tensor(f"ib{s}", [128, 8 * nstream],
                                 mybir.dt.int16, kind="ExternalInput"),
        })

    KPW_MX = max(pl["KPW"] for pl in plan["slots"])
    KPO_MX = max(pl["KPO"] for pl in plan["slots"])

    with tile.TileContext(nc) as tc:
        with (
            tc.tile_pool(name="const", bufs=1) as const,
            tc.tile_pool(name="per_blk", bufs=2) as per_blk,
            tc.tile_pool(name="wpool", bufs=2) as wpool,
            tc.tile_pool(name="tpool", bufs=2) as tpool,
            tc.tile_pool(name="bpool", bufs=2) as bpool,
            tc.tile_pool(name="ipool", bufs=4) as ipool,
            tc.tile_pool(name="small", bufs=4) as small,
            tc.tile_pool(name="psum_t", bufs=2, space="PSUM") as psum_t_pool,
            tc.tile_pool(name="psum_o", bufs=2, space="PSUM") as psum_o_pool,
        ):
            ident = const.tile([128, 128], mybir.dt.float32)
            make_identity(nc, ident[:])
            diag = const.tile([128, 64, 64], mm_dt)
            nc.vector.memset(diag[:].bitcast(mybir.dt.float32), 0.0)
            dg_diag = bass.AP(tensor=diag.tensor, offset=diag[:].offset,
                              ap=[diag[:].ap[0], [65, 64]])

            for s in range(NBLK):
                sl = plan["slots"][s]
                KPW, SW, BW = sl["KPW"], sl["SW"], sl["BW"]
                KPO, SO, BO = sl["KPO"], sl["SO"], sl["BO"]
                stream = sl["stream"]
                din = slot_in[s]

                q_t = per_blk.tile([128, WDP], mybir.dt.float32, tag="q_t")
                nc.sync.dma_start(out=q_t[:], in_=din["q"][:, :])
                mneg_t = per_blk.tile([128, KPW_MX], mybir.dt.float32,
                                      tag="mneg_t")
                nc.sync.dma_start(out=mneg_t[:, :KPW], in_=din["mneg"][:, :KPW])

                # ---- score path ----
                dots = per_blk.tile([128, KPW_MX], mybir.dt.float32, tag="dots")
                for b in range(NBANK):
                    if BW[b] == 0:
                        continue
                    wm_b = wm_d[b * BROWS : (b + 1) * BROWS, :]
                    for j0 in range(0, int(BW[b]), MW):
                        cols = min(MW, int(BW[b]) - j0)
                        gcol = int(SW[b]) + j0
                        n = 128 * cols
                        it = ipool.tile([128, 8 * MW], mybir.dt.int16,
                                        tag="it_w")
                        nc.sync.dma_start(
                            out=it[:, : 8 * cols],
                            in_=din["iw"][:, 8 * gcol : 8 * (gcol + cols)],
                        )
                        w_t = wpool.tile([128, MW, WDP], mybir.dt.float32,
                                         tag="w_t")
                        nc.gpsimd.dma_gather(
                            out_ap=w_t[:, :cols, :], in_ap=wm_b,
                            idxs_ap=it[:, : 8 * cols],
                            num_idxs=n, num_idxs_reg=n, elem_size=WDP,
                            single_packet=False,
                        )
                        prod = tpool.tile([128, MW, WDP], mybir.dt.float32,
                                          tag="prod")
                        nc.vector.tensor_tensor(
                            out=prod[:, :cols, :], in0=w_t[:, :cols, :],
                            in1=q_t[:, None, :].to_broadcast([128, cols, WDP]),
                            op=mybir.AluOpType.mult,
                        )
                        nc.vector.tensor_reduce(
                            out=dots[:, gcol : gcol + cols],
                            in_=prod[:, :cols, :],
                            axis=mybir.AxisListType.X, op=mybir.AluOpType.add,
                        )

                # score -> masked exp -> attn (normalized in token layout)
                nc.vector.tensor_tensor(
                    out=dots[:, :KPW], in0=dots[:, :KPW], in1=mneg_t[:, :KPW],
                    op=mybir.AluOpType.add,
                )
                exp_m = per_blk.tile([128, KPW_MX], mybir.dt.float32,
                                     tag="exp_m")
                denom = small.tile([128, 1], mybir.dt.float32, tag="denom")
                nc.scalar.activation(
                    out=exp_m[:, :KPW], in_=dots[:, :KPW],
                    func=mybir.ActivationFunctionType.Exp, accum_out=denom[:],
                )
                recip = small.tile([128, 1], mybir.dt.float32, tag="recip")
                nc.vector.reciprocal(out=recip[:], in_=denom[:])
                nc.vector.tensor_scalar(
                    out=exp_m[:, :KPW], in0=exp_m[:, :KPW], scalar1=recip[:],
                    scalar2=None, op0=mybir.AluOpType.mult,
                )

                # ---- re-lay attn into 128-aligned bank-chunk axis ----
                attn2 = per_blk.tile([128, KPO_MX], mybir.dt.float32,
                                     tag="attn2")
                nc.vector.memset(attn2[:], 0.0)
                for b in range(NBANK):
                    if BW[b] == 0:
                        continue
                    nc.vector.tensor_copy(
                        out=attn2[:, int(SO[b]) : int(SO[b]) + int(BW[b])],
                        in_=exp_m[:, int(SW[b]) : int(SW[b]) + int(BW[b])],
                    )
                psum_t = psum_t_pool.tile([128, KPO_MX], mybir.dt.float32,
                                          tag="psum_t")
                nchunks = KPO // 128
                for m in range(nchunks):
                    cc = slice(m * 128, (m + 1) * 128)
                    nc.tensor.transpose(out=psum_t[:, cc], in_=attn2[:, cc],
                                        identity=ident[:])
                attn_t = per_blk.tile([128, KPO_MX // 128, 128], mm_dt,
                                      tag="attn_t")
                nc.scalar.copy(
                    out=attn_t[:, :nchunks, :].rearrange("p a b -> p (a b)"),
                    in_=psum_t[:, : nchunks * 128],
                )

                # ---- output path: real columns only ----
                doc_ps = [
                    psum_o_pool.tile([64, HDP], mybir.dt.float32,
                                     tag=f"doc_ps{g}", name=f"doc_ps{g}_{s}")
                    for g in range(2)
                ]
                first = [True, True]
                last_j = [max(j for j, (_b, g, _c, _t) in enumerate(stream)
                              if g == gg) for gg in range(2)]
                cur_diag = None
                j = 0
                while j < len(stream):
                    jend = min(j + MBC, len(stream))
                    # keep chunk within one bank
                    b0 = stream[j][0]
                    while jend > j + 1 and stream[jend - 1][0] != b0:
                        jend -= 1
                    cols = jend - j
                    n = 128 * cols
                    it = ipool.tile([128, 8 * MBC], mybir.dt.int16, tag="it_b")
                    nc.sync.dma_start(
                        out=it[:, : 8 * cols],
                        in_=din["ib"][:, 8 * j : 8 * jend],
                    )
                    b_t = bpool.tile([128, MBC, HDP], bm_dt, tag="b_t")
                    nc.gpsimd.dma_gather(
                        out_ap=b_t[:, :cols, :],
                        in_ap=bm_d[b0 * BROWS : (b0 + 1) * BROWS, :],
                        idxs_ap=it[:, : 8 * cols],
                        num_idxs=n, num_idxs_reg=n, elem_size=HDP,
                        single_packet=False,
                    )
                    for jj in range(j, jend):
                        b, g, cc, tt = stream[jj]
                        m = int(SO[b]) // 128 + cc
                        if cur_diag != (m, g):
                            nc.scalar.copy(
                                out=dg_diag,
                                in_=attn_t[:, m, g * 64 : g * 64 + 64],
                            )
                            cur_diag = (m, g)
                        nc.tensor.matmul(
                            out=doc_ps[g][:],
                            lhsT=diag[:, tt, :],
                            rhs=b_t[:, jj - j, :],
                            start=first[g],
                            stop=(jj == last_j[g]),
                        )
                        first[g] = False
                    j = jend

                for g in range(2):
                    doc_sb = per_blk.tile([64, HDP], mybir.dt.float32,
                                          tag=f"doc_sb{g}", name=f"dsb{g}_{s}")
                    nc.scalar.copy(out=doc_sb[:], in_=doc_ps[g][:])
                    nc.sync.dma_start(
                        out=doc_d[s * 128 + g * 64 : s * 128 + g * 64 + 64, :],
                        in_=doc_sb[:],
                    )

    nc.compile()
    return nc


# --------------------------------------------------------------------------
# entry point
# --------------------------------------------------------------------------

def _sig(plan):
    parts = []
    for sl in plan["slots"]:
        parts.append(tuple(int(x) for x in sl["BW"]))
        parts.append(tuple(int(x) for x in sl["BO"]))
        parts.append(len(sl["stream"]))
    return tuple(parts)


def kernel(word_embs, wordmem, bankmem, idx, lengths, _trace=False, **_kw):
    we = np.asarray(word_embs, dtype=np.float32)
    wm = np.asarray(wordmem, dtype=np.float32)
    bm = np.asarray(bankmem, dtype=np.float32)

    plan = _plan(idx, lengths)
    sig = _sig(plan)
    if _CACHE.get("sig") != sig:
        _CACHE["nc"] = _build_nc(plan)
        _CACHE["sig"] = sig
    nc = _CACHE["nc"]

    # host-normalized, padded tables (same f32 math as the reference)
    wnorm = np.sqrt((wm * wm).sum(axis=1, dtype=np.float32).astype(np.float32))
    wn = wm / np.maximum(wnorm, np.float32(1e-12))[:, None]
    wm2 = np.zeros((V, WDP), dtype=np.float32)
    wm2[:, :WD] = wn
    bm2 = np.zeros((V, HDP), dtype=np.float32)
    bm2[:, :HD] = bm
    qnorm = np.sqrt((we * we).sum(axis=1, dtype=np.float32).astype(np.float32))
    we_hat = we / np.maximum(qnorm, np.float32(1e-12))[:, None]

    in_maps = []
    for c in range(N_CORES):
        m = _per_core_arrays(plan, we_hat, c)
        m["wm"] = wm2
        m["bm"] = bm2
        in_maps.append(m)

    kw = {"trace": True, "trace_cores": [0]} if _trace else {}
    res = run_bass_kernel_spmd(nc, in_maps, core_ids=list(range(N_CORES)), **kw)
    if _trace:
        print(f"HW exec time: {res.exec_time_ns} ns")
        _CACHE["last_trace"] = res

    out = np.zeros((NUM, HD), dtype=np.float32)
    for c in range(N_CORES):
        doc = res.results[c]["doc"]
        for s in range(NBLK):
            blk = plan["blocks"][plan["core_blocks"][c][s]]
            out[blk] = doc[s * 128 : (s + 1) * 128, :HD]
    return out



# revision 2
# speedup vs baseline: 4.1899x; 4.1899x over previous
"""MemoryBank scatter-gather kernel for 8 Trainium2 NeuronCores.

Reference (per token n of 2048, K=500 neighbor slots, len=lengths[n]):
    neigh = l2norm(wordmem[idx[n,:len]]); q = l2norm(word_embs[n])
    score = q @ neigh.T ; attn = softmax-over-valid(score)
    out   = attn @ bankmem[idx[n,:len]]

Design (v2 — minimize gathered rows; GpSimd desc-gen is ~8ns/row serial):
  * ONE combined bf16 table row per bank row: [w_hat(100) | pad | 1.0 at
    col 111 | bank(400)] = 512 bf16 = 1024B (256B-aligned for dma_gather).
    Each (token, neighbor) pair costs exactly one gathered row instead of
    separate wordmem + bankmem fetches: 64k rows/core vs 240k baseline.
  * Tokens length-sorted and snake-dealt into 16 groups of 128
    (core = g%8, slot = g//8) so per-core pair counts balance.  Within a
    (slot, bank) all pairs are packed token-major into 128-row gather
    columns; the program shape depends only on per-(slot,bank) column
    counts (max across cores, cached by signature).
  * Scores: host streams replicated q-hat rows (bf16, sequential DMA, no
    gather); DVE mult+reduce against the w part of the gathered rows.
  * attn accumulation on PE: per column j, lhsT[p,t] = (iota[t]==tokid[p])
    * exp(score[p]) built by one DVE tensor_scalar; matmul with
    rhs = gathered[:, j, 111:512] accumulates [128 tok, 1+400] in PSUM —
    column 0 (the table's constant 1.0) is the softmax denominator, so
    normalization is one reciprocal + scale at slot end.  Pad slots get
    tokid=-1 => lhsT row 0 => no contribution to doc or denom.
"""

import numpy as np
import ml_dtypes

import concourse.bacc as bacc
import concourse.mybir as mybir
import concourse.tile as tile
from concourse.bass_utils import run_bass_kernel_spmd

BF16 = ml_dtypes.bfloat16

NUM = 2048
K = 500
WD = 100
HD = 400
V = 100000
N_CORES = 8
NSLOT = 2                      # 2 groups of 128 tokens per core
NBANK = 4
BROWS = V // NBANK             # 25000 rows per bank (< int16 max)
TE = 512                       # combined row elems (bf16) = 1024B
ONE_POS = 111                  # constant 1.0 column (denominator trick)
RHS_W = 1 + HD                 # matmul rhs width: [1.0 | bank row]
QE = 128                       # replicated q-hat row elems (w part width)
CHUNK = 16                     # gather columns per instruction (2048 rows)

_CACHE: dict = {}


# --------------------------------------------------------------------------
# host planning
# --------------------------------------------------------------------------

def _wrap16(flat_i16):
    """i-th index -> (i%16, i//16), replicated to 128 partitions."""
    n = flat_i16.shape[0]
    assert n % 16 == 0
    blk = flat_i16.reshape(-1, 16).T.copy()            # [16, n/16]
    return np.tile(blk, (8, 1))                        # [128, n/16]


def _plan(idx, lengths):
    idx = np.asarray(idx, dtype=np.int64)
    lengths = np.asarray(lengths, dtype=np.int64)

    order = np.argsort(-lengths, kind="stable")
    g = order.reshape(K // 4 * 0 + NUM // 16, 16).copy()   # [128 rounds, 16]
    g[1::2] = g[1::2, ::-1]                                # snake deal
    groups = g.T.copy()                                    # [16, 128] token ids

    # per (group, bank): token-major packed local indices + owning position
    seg_loc = [[None] * NBANK for _ in range(16)]
    seg_pos = [[None] * NBANK for _ in range(16)]
    cnt = np.zeros((16, NBANK), dtype=np.int64)
    for j in range(16):
        locs = [[] for _ in range(NBANK)]
        poss = [[] for _ in range(NBANK)]
        for p in range(128):
            t = groups[j][p]
            v = idx[t, : lengths[t]]
            b = v // BROWS
            for bb in range(NBANK):
                lv = v[b == bb] - bb * BROWS
                locs[bb].append(lv)
                poss[bb].append(np.full(lv.shape[0], p, dtype=np.int64))
        for bb in range(NBANK):
            seg_loc[j][bb] = np.concatenate(locs[bb])
            seg_pos[j][bb] = np.concatenate(poss[bb])
            cnt[j][bb] = seg_loc[j][bb].shape[0]

    # shared program shape: per (slot, bank) column count = max over cores
    ncol = np.zeros((NSLOT, NBANK), dtype=np.int64)
    for s in range(NSLOT):
        for b in range(NBANK):
            cs = cnt[[s * 8 + c for c in range(N_CORES)], b]
            ncol[s][b] = -(-int(cs.max()) // 128)
    return {"groups": groups, "seg_loc": seg_loc, "seg_pos": seg_pos,
            "cnt": cnt, "ncol": ncol}


def _per_core_arrays(plan, we_hat16, core):
    """Build one core's iw / tokid / q arrays for both slots."""
    ncol = plan["ncol"]
    out = {}
    for s in range(NSLOT):
        j = s * 8 + core
        NC = int(ncol[s].sum())
        iw_segs = []
        tokid = np.full((NC, 128), -1.0, dtype=np.float32)
        qpk = np.zeros((NC, 128, QE), dtype=BF16)
        c0 = 0
        for b in range(NBANK):
            nb = int(ncol[s][b])
            if nb == 0:
                continue
            npair = nb * 128
            loc = np.zeros(npair, dtype=np.int16)
            real = plan["seg_loc"][j][b]
            pos = plan["seg_pos"][j][b]
            n = real.shape[0]
            loc[:n] = real.astype(np.int16)
            iw_segs.append(_wrap16(loc))
            tk = tokid[c0 : c0 + nb].reshape(-1)
            tk[:n] = pos.astype(np.float32)
            qp = qpk[c0 : c0 + nb].reshape(npair, QE)
            qp[:n] = we_hat16[plan["groups"][j][pos]]
            c0 += nb
        out[f"iw{s}"] = np.concatenate(iw_segs, axis=1)
        out[f"tokid{s}"] = np.ascontiguousarray(tokid.T)            # [128, NC]
        out[f"q{s}"] = np.ascontiguousarray(qpk.transpose(1, 0, 2))  # [128,NC,QE]
    return out


# --------------------------------------------------------------------------
# bass program (built per column-count signature)
# --------------------------------------------------------------------------

def _build_nc(ncol):
    nc = bacc.Bacc(None, target_bir_lowering=False)
    bf = mybir.dt.bfloat16
    f32 = mybir.dt.float32

    tbl_d = nc.dram_tensor("tbl", [V, TE], bf, kind="ExternalInput")
    iota_d = nc.dram_tensor("iota", [128, 128], f32, kind="ExternalInput")
    doc_d = nc.dram_tensor("doc", [NSLOT * 128, HD], f32,
                           kind="ExternalOutput")
    slot_in = []
    for s in range(NSLOT):
        NC = int(ncol[s].sum())
        slot_in.append({
            "q": nc.dram_tensor(f"q{s}", [128, NC, QE], bf,
                                kind="ExternalInput"),
            "tokid": nc.dram_tensor(f"tokid{s}", [128, NC], f32,
                                    kind="ExternalInput"),
            "iw": nc.dram_tensor(f"iw{s}", [128, 8 * NC], mybir.dt.int16,
                                 kind="ExternalInput"),
        })

    NC_MX = max(int(ncol[s].sum()) for s in range(NSLOT))

    with tile.TileContext(nc) as tc:
        with (
            tc.tile_pool(name="const", bufs=1) as const,
            tc.tile_pool(name="per_slot", bufs=2) as per_slot,
            tc.tile_pool(name="gpool", bufs=3) as gpool,
            tc.tile_pool(name="qpool", bufs=3) as qpool,
            tc.tile_pool(name="ppool", bufs=2) as ppool,
            tc.tile_pool(name="lpool", bufs=2) as lpool,
            tc.tile_pool(name="small", bufs=4) as small,
            tc.tile_pool(name="psum_o", bufs=2, space="PSUM") as psum_o_pool,
        ):
            iota_t = const.tile([128, 128], f32)
            nc.sync.dma_start(out=iota_t[:], in_=iota_d[:, :])

            for s in range(NSLOT):
                NC = int(ncol[s].sum())
                din = slot_in[s]

                iw_t = per_slot.tile([128, 8 * NC_MX], mybir.dt.int16,
                                     tag="iw_t")
                nc.sync.dma_start(out=iw_t[:, : 8 * NC], in_=din["iw"][:, :])
                tokid_t = per_slot.tile([128, NC_MX], f32, tag="tokid_t")
                nc.sync.dma_start(out=tokid_t[:, :NC], in_=din["tokid"][:, :])

                psum_t = psum_o_pool.tile([128, RHS_W], f32, tag="psum_t",
                                          name=f"psum_{s}")

                # chunk list: (bank, global col, cols) within one bank each
                chunks = []
                c0 = 0
                for b in range(NBANK):
                    nb = int(ncol[s][b])
                    for cc in range(0, nb, CHUNK):
                        chunks.append((b, c0 + cc, min(CHUNK, nb - cc)))
                    c0 += nb
                last = len(chunks) - 1

                for ci, (b, gc, cols) in enumerate(chunks):
                    n = 128 * cols
                    g_t = gpool.tile([128, CHUNK, TE], bf, tag="g_t")
                    nc.gpsimd.dma_gather(
                        out_ap=g_t[:, :cols, :],
                        in_ap=tbl_d[b * BROWS : (b + 1) * BROWS, :],
                        idxs_ap=iw_t[:, 8 * gc : 8 * (gc + cols)],
                        num_idxs=n, num_idxs_reg=n, elem_size=TE,
                        single_packet=False,
                    )
                    q_t = qpool.tile([128, CHUNK, QE], bf, tag="q_t")
                    nc.sync.dma_start(out=q_t[:, :cols, :],
                                      in_=din["q"][:, gc : gc + cols, :])
                    prod = ppool.tile([128, CHUNK, QE], f32, tag="prod")
                    nc.vector.tensor_tensor(
                        out=prod[:, :cols, :], in0=g_t[:, :cols, 0:QE],
                        in1=q_t[:, :cols, :], op=mybir.AluOpType.mult,
                    )
                    scores = small.tile([128, CHUNK], f32, tag="scores")
                    nc.vector.tensor_reduce(
                        out=scores[:, :cols], in_=prod[:, :cols, :],
                        axis=mybir.AxisListType.X, op=mybir.AluOpType.add,
                    )
                    exps = small.tile([128, CHUNK], f32, tag="exps")
                    nc.scalar.activation(
                        out=exps[:, :cols], in_=scores[:, :cols],
                        func=mybir.ActivationFunctionType.Exp,
                    )
                    lhsT = lpool.tile([128, CHUNK, 128], bf, tag="lhsT")
                    for j in range(cols):
                        nc.vector.tensor_scalar(
                            out=lhsT[:, j, :], in0=iota_t[:],
                            scalar1=tokid_t[:, gc + j : gc + j + 1],
                            scalar2=exps[:, j : j + 1],
                            op0=mybir.AluOpType.is_equal,
                            op1=mybir.AluOpType.mult,
                        )
                        nc.tensor.matmul(
                            out=psum_t[:],
                            lhsT=lhsT[:, j, :],
                            rhs=g_t[:, j, ONE_POS : ONE_POS + RHS_W],
                            start=(ci == 0 and j == 0),
                            stop=(ci == last and j == cols - 1),
                        )

                recip = small.tile([128, 1], f32, tag="recip",
                                   name=f"recip_{s}")
                nc.vector.reciprocal(out=recip[:], in_=psum_t[:, 0:1])
                doc_sb = per_slot.tile([128, HD], f32, tag="doc_sb")
                nc.vector.tensor_scalar(
                    out=doc_sb[:], in0=psum_t[:, 1:RHS_W], scalar1=recip[:],
                    scalar2=None, op0=mybir.AluOpType.mult,
                )
                nc.sync.dma_start(out=doc_d[s * 128 : (s + 1) * 128, :],
                                  in_=doc_sb[:])

    nc.compile()
    return nc


# --------------------------------------------------------------------------
# entry point
# --------------------------------------------------------------------------

def kernel(word_embs, wordmem, bankmem, idx, lengths, _trace=False, **_kw):
    we = np.asarray(word_embs, dtype=np.float32)
    wm = np.asarray(wordmem, dtype=np.float32)
    bm = np.asarray(bankmem, dtype=np.float32)

    plan = _plan(idx, lengths)
    sig = tuple(int(x) for x in plan["ncol"].reshape(-1))
    if _CACHE.get("sig") != sig:
        _CACHE["nc"] = _build_nc(plan["ncol"])
        _CACHE["sig"] = sig
    nc = _CACHE["nc"]

    # combined bf16 table: [w_hat | 0-pad | 1.0 | bank], 512 elems = 1024B
    wnorm = np.sqrt((wm * wm).sum(axis=1, dtype=np.float32))
    wn = wm / np.maximum(wnorm, np.float32(1e-12))[:, None]
    tbl = np.zeros((V, TE), dtype=BF16)
    tbl[:, :WD] = wn.astype(BF16)
    tbl[:, ONE_POS] = np.float32(1.0)
    tbl[:, ONE_POS + 1 : ONE_POS + 1 + HD] = bm.astype(BF16)

    qnorm = np.sqrt((we * we).sum(axis=1, dtype=np.float32))
    we_hat = we / np.maximum(qnorm, np.float32(1e-12))[:, None]
    we_hat16 = np.zeros((NUM, QE), dtype=BF16)
    we_hat16[:, :WD] = we_hat.astype(BF16)

    iota = np.tile(np.arange(128, dtype=np.float32)[None, :], (128, 1))

    in_maps = []
    for c in range(N_CORES):
        m = _per_core_arrays(plan, we_hat16, c)
        m["tbl"] = tbl
        m["iota"] = iota
        in_maps.append(m)

    kw = {"trace": True, "trace_cores": [0]} if _trace else {}
    res = run_bass_kernel_spmd(nc, in_maps, core_ids=list(range(N_CORES)), **kw)
    if _trace:
        print(f"HW exec time: {res.exec_time_ns} ns")
        _CACHE["last_trace"] = res

    out = np.zeros((NUM, HD), dtype=np.float32)
    for c in range(N_CORES):
        doc = res.results[c]["doc"]
        for s in range(NSLOT):
            out[plan["groups"][s * 8 + c]] = doc[s * 128 : (s + 1) * 128, :]
    return out


# revision 9
# speedup vs baseline: 5.2369x; 1.2499x over previous
"""MemoryBank scatter-gather kernel for 8 Trainium2 NeuronCores.

Reference (per token n of 2048, K=500 neighbor slots, len=lengths[n]):
    neigh = l2norm(wordmem[idx[n,:len]]); q = l2norm(word_embs[n])
    score = q @ neigh.T ; attn = softmax-over-valid(score)
    out   = attn @ bankmem[idx[n,:len]]

Design (v2 — minimize gathered rows; GpSimd desc-gen is ~8ns/row serial):
  * ONE combined bf16 table row per bank row: [w_hat(100) | pad | 1.0 at
    col 111 | bank(400)] = 512 bf16 = 1024B (256B-aligned for dma_gather).
    Each (token, neighbor) pair costs exactly one gathered row instead of
    separate wordmem + bankmem fetches: 64k rows/core vs 240k baseline.
  * Tokens length-sorted and snake-dealt into 16 groups of 128
    (core = g%8, slot = g//8) so per-core pair counts balance.  Within a
    (slot, bank) all pairs are packed token-major into 128-row gather
    columns; the program shape depends only on per-(slot,bank) column
    counts (max across cores, cached by signature).
  * Scores: host streams replicated q-hat rows (bf16, sequential DMA, no
    gather); DVE mult+reduce against the w part of the gathered rows.
  * attn accumulation on PE: per column j, lhsT[p,t] = (iota[t]==tokid[p])
    * exp(score[p]) built by one DVE tensor_scalar; matmul with
    rhs = gathered[:, j, 111:512] accumulates [128 tok, 1+400] in PSUM —
    column 0 (the table's constant 1.0) is the softmax denominator, so
    normalization is one reciprocal + scale at slot end.  Pad slots get
    tokid=-1 => lhsT row 0 => no contribution to doc or denom.
"""

import numpy as np
import ml_dtypes

import concourse.bacc as bacc
import concourse.mybir as mybir
import concourse.tile as tile
from concourse.bass_utils import run_bass_kernel_spmd

BF16 = ml_dtypes.bfloat16

NUM = 2048
K = 500
WD = 100
HD = 400
V = 100000
N_CORES = 8
NSLOT = 2                      # 2 groups of 128 tokens per core
NBANK = 4
BROWS = V // NBANK             # 25000 rows per bank (< int16 max)
TE = 512                       # combined row elems (bf16) = 1024B
ONE_POS = 111                  # constant 1.0 column (denominator trick)
RHS_W = 1 + HD                 # matmul rhs width: [1.0 | bank row]
QE = 128                       # replicated q-hat row elems (w part width)
CHUNK = 16                     # gather columns per instruction (2048 rows)

_CACHE: dict = {}


# --------------------------------------------------------------------------
# host planning
# --------------------------------------------------------------------------

def _wrap16(flat_i16):
    """i-th index -> (i%16, i//16), replicated to 128 partitions."""
    n = flat_i16.shape[0]
    assert n % 16 == 0
    blk = flat_i16.reshape(-1, 16).T.copy()            # [16, n/16]
    return np.tile(blk, (8, 1))                        # [128, n/16]


def _plan(idx, lengths):
    idx = np.asarray(idx, dtype=np.int64)
    lengths = np.asarray(lengths, dtype=np.int64)

    order = np.argsort(-lengths, kind="stable")
    g = order.reshape(K // 4 * 0 + NUM // 16, 16).copy()   # [128 rounds, 16]
    g[1::2] = g[1::2, ::-1]                                # snake deal
    groups = g.T.copy()                                    # [16, 128] token ids

    # per (group, bank): token-major packed local indices + owning position
    seg_loc = [[None] * NBANK for _ in range(16)]
    seg_pos = [[None] * NBANK for _ in range(16)]
    cnt = np.zeros((16, NBANK), dtype=np.int64)
    for j in range(16):
        locs = [[] for _ in range(NBANK)]
        poss = [[] for _ in range(NBANK)]
        for p in range(128):
            t = groups[j][p]
            v = idx[t, : lengths[t]]
            b = v // BROWS
            for bb in range(NBANK):
                lv = v[b == bb] - bb * BROWS
                locs[bb].append(lv)
                poss[bb].append(np.full(lv.shape[0], p, dtype=np.int64))
        for bb in range(NBANK):
            seg_loc[j][bb] = np.concatenate(locs[bb])
            seg_pos[j][bb] = np.concatenate(poss[bb])
            cnt[j][bb] = seg_loc[j][bb].shape[0]

    # shared program shape: per (slot, bank) column count = max over cores
    ncol = np.zeros((NSLOT, NBANK), dtype=np.int64)
    for s in range(NSLOT):
        for b in range(NBANK):
            cs = cnt[[s * 8 + c for c in range(N_CORES)], b]
            ncol[s][b] = -(-int(cs.max()) // 128)
    return {"groups": groups, "seg_loc": seg_loc, "seg_pos": seg_pos,
            "cnt": cnt, "ncol": ncol}


def _per_core_arrays(plan, we_hat16, core):
    """Build one core's iw / tokid / q arrays for both slots."""
    ncol = plan["ncol"]
    out = {}
    for s in range(NSLOT):
        j = s * 8 + core
        NC = int(ncol[s].sum())
        iw_segs = []
        tokid = np.full((NC, 128), -1.0, dtype=BF16)
        qpk = np.zeros((NC, 128, QE), dtype=BF16)
        c0 = 0
        for b in range(NBANK):
            nb = int(ncol[s][b])
            if nb == 0:
                continue
            npair = nb * 128
            loc = np.zeros(npair, dtype=np.int16)
            real = plan["seg_loc"][j][b]
            pos = plan["seg_pos"][j][b]
            n = real.shape[0]
            loc[:n] = real.astype(np.int16)
            iw_segs.append(_wrap16(loc))
            tk = tokid[c0 : c0 + nb].reshape(-1)
            tk[:n] = pos.astype(BF16)
            qp = qpk[c0 : c0 + nb].reshape(npair, QE)
            qp[:n] = we_hat16[plan["groups"][j][pos]]
            c0 += nb
        out[f"iw{s}"] = np.concatenate(iw_segs, axis=1)
        out[f"tokid{s}"] = np.ascontiguousarray(tokid.T)            # [128, NC]
        out[f"q{s}"] = np.ascontiguousarray(qpk.transpose(1, 0, 2))  # [128,NC,QE]
    return out


# --------------------------------------------------------------------------
# bass program (built per column-count signature)
# --------------------------------------------------------------------------

def _build_nc(ncol):
    nc = bacc.Bacc(None, target_bir_lowering=False)
    bf = mybir.dt.bfloat16
    f32 = mybir.dt.float32

    tbl_d = nc.dram_tensor("tbl", [V, TE], bf, kind="ExternalInput")
    # iota3[p, t, j] = t — materialized so the eq build needs no
    # innermost-stride-0 broadcast
    iota_d = nc.dram_tensor("iota", [128, 128, CHUNK], bf,
                            kind="ExternalInput")
    doc_d = nc.dram_tensor("doc", [NSLOT * 128, HD], f32,
                           kind="ExternalOutput")
    slot_in = []
    for s in range(NSLOT):
        NC = int(ncol[s].sum())
        slot_in.append({
            "q": nc.dram_tensor(f"q{s}", [128, NC, QE], bf,
                                kind="ExternalInput"),
            "tokid": nc.dram_tensor(f"tokid{s}", [128, NC], bf,
                                    kind="ExternalInput"),
            "iw": nc.dram_tensor(f"iw{s}", [128, 8 * NC], mybir.dt.int16,
                                 kind="ExternalInput"),
        })

    NC_MX = max(int(ncol[s].sum()) for s in range(NSLOT))

    with tile.TileContext(nc) as tc:
        with (
            tc.tile_pool(name="const", bufs=1) as const,
            tc.tile_pool(name="per_slot", bufs=2) as per_slot,
            tc.tile_pool(name="gpool", bufs=3) as gpool,
            tc.tile_pool(name="qpool", bufs=3) as qpool,
            tc.tile_pool(name="ppool", bufs=2) as ppool,
            tc.tile_pool(name="lpool", bufs=2) as lpool,
            tc.tile_pool(name="small", bufs=4) as small,
            tc.tile_pool(name="psum_o", bufs=2, space="PSUM") as psum_o_pool,
        ):
            iota_t = const.tile([128, 128, CHUNK], bf)
            nc.sync.dma_start(out=iota_t[:], in_=iota_d[:, :, :])

            for s in range(NSLOT):
                NC = int(ncol[s].sum())
                din = slot_in[s]

                iw_t = per_slot.tile([128, 8 * NC_MX], mybir.dt.int16,
                                     tag="iw_t")
                nc.sync.dma_start(out=iw_t[:, : 8 * NC], in_=din["iw"][:, :])
                tokid_t = per_slot.tile([128, NC_MX], bf, tag="tokid_t")
                nc.sync.dma_start(out=tokid_t[:, :NC], in_=din["tokid"][:, :])

                psum_t = psum_o_pool.tile([128, RHS_W], f32, tag="psum_t",
                                          name=f"psum_{s}")

                # chunk list: (bank, global col, cols) within one bank each
                chunks = []
                c0 = 0
                for b in range(NBANK):
                    nb = int(ncol[s][b])
                    for cc in range(0, nb, CHUNK):
                        chunks.append((b, c0 + cc, min(CHUNK, nb - cc)))
                    c0 += nb
                last = len(chunks) - 1

                for ci, (b, gc, cols) in enumerate(chunks):
                    n = 128 * cols
                    g_t = gpool.tile([128, CHUNK, TE], bf, tag="g_t")
                    nc.gpsimd.dma_gather(
                        out_ap=g_t[:, :cols, :],
                        in_ap=tbl_d[b * BROWS : (b + 1) * BROWS, :],
                        idxs_ap=iw_t[:, 8 * gc : 8 * (gc + cols)],
                        num_idxs=n, num_idxs_reg=n, elem_size=TE,
                        single_packet=False,
                    )
                    q_t = qpool.tile([128, CHUNK, QE], bf, tag="q_t")
                    nc.sync.dma_start(out=q_t[:, :cols, :],
                                      in_=din["q"][:, gc : gc + cols, :])
                    prod = ppool.tile([128, CHUNK, QE], bf, tag="prod")
                    nc.vector.tensor_tensor(
                        out=prod[:, :cols, :], in0=g_t[:, :cols, 0:QE],
                        in1=q_t[:, :cols, :], op=mybir.AluOpType.mult,
                    )
                    scores = small.tile([128, CHUNK], f32, tag="scores")
                    nc.vector.tensor_reduce(
                        out=scores[:, :cols], in_=prod[:, :cols, :],
                        axis=mybir.AxisListType.X, op=mybir.AluOpType.add,
                    )
                    exps = small.tile([128, CHUNK], bf, tag="exps")
                    nc.scalar.activation(
                        out=exps[:, :cols], in_=scores[:, :cols],
                        func=mybir.ActivationFunctionType.Exp,
                    )
                    # lhsT_t[p, t, j] = (t == tokid[p, gc+j]) * exp[p, j]
                    eq_t = lpool.tile([128, 128, CHUNK], bf, tag="eq_t")
                    nc.vector.tensor_tensor(
                        out=eq_t[:, :, :cols], in0=iota_t[:, :, :cols],
                        in1=tokid_t[:, None, gc : gc + cols].to_broadcast(
                            [128, 128, cols]),
                        op=mybir.AluOpType.is_equal,
                    )
                    lhsT = lpool.tile([128, 128, CHUNK], bf, tag="lhsT")
                    nc.vector.tensor_tensor(
                        out=lhsT[:, :, :cols], in0=eq_t[:, :, :cols],
                        in1=exps[:, None, :cols].to_broadcast([128, 128, cols]),
                        op=mybir.AluOpType.mult,
                    )
                    for j in range(cols):
                        nc.tensor.matmul(
                            out=psum_t[:],
                            lhsT=lhsT[:, :, j],
                            rhs=g_t[:, j, ONE_POS : ONE_POS + RHS_W],
                            start=(ci == 0 and j == 0),
                            stop=(ci == last and j == cols - 1),
                        )

                recip = small.tile([128, 1], f32, tag="recip",
                                   name=f"recip_{s}")
                nc.vector.reciprocal(out=recip[:], in_=psum_t[:, 0:1])
                doc_sb = per_slot.tile([128, HD], f32, tag="doc_sb")
                nc.vector.tensor_scalar(
                    out=doc_sb[:], in0=psum_t[:, 1:RHS_W], scalar1=recip[:],
                    scalar2=None, op0=mybir.AluOpType.mult,
                )
                nc.sync.dma_start(out=doc_d[s * 128 : (s + 1) * 128, :],
                                  in_=doc_sb[:])

    nc.compile()
    return nc


# --------------------------------------------------------------------------
# entry point
# --------------------------------------------------------------------------

def kernel(word_embs, wordmem, bankmem, idx, lengths, _trace=False, **_kw):
    we = np.asarray(word_embs, dtype=np.float32)
    wm = np.asarray(wordmem, dtype=np.float32)
    bm = np.asarray(bankmem, dtype=np.float32)

    plan = _plan(idx, lengths)
    sig = tuple(int(x) for x in plan["ncol"].reshape(-1))
    if _CACHE.get("sig") != sig:
        _CACHE["nc"] = _build_nc(plan["ncol"])
        _CACHE["sig"] = sig
    nc = _CACHE["nc"]

    # combined bf16 table: [w_hat | 0-pad | 1.0 | bank], 512 elems = 1024B
    wnorm = np.sqrt((wm * wm).sum(axis=1, dtype=np.float32))
    wn = wm / np.maximum(wnorm, np.float32(1e-12))[:, None]
    tbl = np.zeros((V, TE), dtype=BF16)
    tbl[:, :WD] = wn.astype(BF16)
    tbl[:, ONE_POS] = np.float32(1.0)
    tbl[:, ONE_POS + 1 : ONE_POS + 1 + HD] = bm.astype(BF16)

    qnorm = np.sqrt((we * we).sum(axis=1, dtype=np.float32))
    we_hat = we / np.maximum(qnorm, np.float32(1e-12))[:, None]
    we_hat16 = np.zeros((NUM, QE), dtype=BF16)
    we_hat16[:, :WD] = we_hat.astype(BF16)

    iota = np.ascontiguousarray(np.broadcast_to(
        np.arange(128, dtype=np.float32)[None, :, None].astype(BF16),
        (128, 128, CHUNK)))

    in_maps = []
    for c in range(N_CORES):
        m = _per_core_arrays(plan, we_hat16, c)
        m["tbl"] = tbl
        m["iota"] = iota
        in_maps.append(m)

    kw = {"trace": True, "trace_cores": [0]} if _trace else {}
    res = run_bass_kernel_spmd(nc, in_maps, core_ids=list(range(N_CORES)), **kw)
    if _trace:
        print(f"HW exec time: {res.exec_time_ns} ns")
        _CACHE["last_trace"] = res

    out = np.zeros((NUM, HD), dtype=np.float32)
    for c in range(N_CORES):
        doc = res.results[c]["doc"]
        for s in range(NSLOT):
            out[plan["groups"][s * 8 + c]] = doc[s * 128 : (s + 1) * 128, :]
    return out
